# revision 1
# baseline (speedup 1.0000x reference)
"""EntropyByteLatentTransformer Trainium2 kernel (8 NeuronCores).

Sharding: batch b = core//2, sequence half s = core%2 (1024 tokens/core).
Per layer, each core computes q/k/v for its tokens; k (feature-major) and v
(token-major) are exchanged within the core pair via AllGather so every core
attends over the full 2048-key sequence for its 1024 queries.

Layout: activations are feature-major (hT [D, tok], D chunked 4x128 on
partitions). Matmuls run in fp32r (full-rate fp32, ~13-bit-mantissa input
rounding) with weights pre-transposed on the host. Softmax runs without
max-subtraction (scores are O(1) by construction); the denominator comes free
from a ones-column appended to v in the attn@v matmul (lands on psum partition
64). LayerNorm statistics are partition-dim reductions on the tensor engine via
a ones column; rsqrt(var) = exp(-0.5*ln(var+eps)) keeps the scalar engine on
one activation-table set (ln/exp) through LN + attention.
"""
import numpy as np
from functools import lru_cache

import concourse.bass as bass
import concourse.tile as tile
import concourse.mybir as mybir
from concourse import bacc
from concourse.bass_utils import run_bass_kernel_spmd

B, L, D, H, NL, V, W = 4, 2048, 512, 8, 4, 258, 8
HD = D // H          # 64
P = 128
TL = L // 2          # 1024 tokens per core
KC = D // P          # 4 feature chunks
MC = L // P          # 16 key chunks (full seq)
NCORE = 8

f32 = mybir.dt.float32
f32r = mybir.dt.float32r
i32 = mybir.dt.int32
EXP = mybir.ActivationFunctionType.Exp
LN_ = mybir.ActivationFunctionType.Ln
GELU = mybir.ActivationFunctionType.Gelu
ADD = mybir.AluOpType.add
SUB = mybir.AluOpType.subtract
MUL = mybir.AluOpType.mult
EQ = mybir.AluOpType.is_equal
LT = mybir.AluOpType.is_lt

DEBUG_TAPS = False
SIM_MODE = False  # replace collectives with local copies (single-core TimelineSim)
NO_COLLECTIVE = False  # 8-core run but collectives replaced by local DMA (timing ablation)

INV_8LN2 = float(1.0 / (8.0 * np.log(2.0)))
ATT_SCALE = float(1.0 / np.sqrt(HD))


def _ap(base_ap, offset_elems, pairs):
    """Raw AP over the same tensor with explicit [step, count] pairs."""
    return bass.AP(base_ap.tensor, base_ap.offset + offset_elems, pairs)


def _row_dma(nc, dst_tile, src_dram_ap):
    """DMA a DRAM tensor (any shape) into a 1-partition SBUF tile row."""
    flat = src_dram_ap
    while flat.ndim > 1:
        flat = flat.rearrange(
            " ".join(f"d{i}" for i in range(flat.ndim)) + " -> "
            + "(" + " ".join(f"d{i}" for i in range(flat.ndim)) + ")")
    n = flat.shape[0]
    nc.sync.dma_start(dst_tile[:], flat.unsqueeze(0))


@lru_cache(maxsize=None)
def build_program(repeat=1):
    nc = bacc.Bacc("TRN2", target_bir_lowering=False, debug=False,
                   num_devices=1 if SIM_MODE else NCORE)
    taps = {}

    def tap(name, ap_fn, shape):
        if not DEBUG_TAPS:
            return None
        out = nc.dram_tensor("tap_" + name, shape, f32, kind="ExternalOutput").ap()
        taps[name] = (out, ap_fn)
        return out
    build_program.taps = taps

    emb_in = nc.dram_tensor("emb", [V, D], f32, kind="ExternalInput").ap()
    ident_in = nc.dram_tensor("ident", [P, P], f32, kind="ExternalInput").ap()
    xidx_in = nc.dram_tensor("x_idx", [TL], i32, kind="ExternalInput").ap()
    xent_in = nc.dram_tensor("x_ent", [1032], f32, kind="ExternalInput").ap()
    emask_in = nc.dram_tensor("ent_mask", [TL], f32, kind="ExternalInput").ap()
    entw_in = nc.dram_tensor("entw_row", [KC * P], f32r, kind="ExternalInput").ap()
    entb_in = nc.dram_tensor("entb_row", [KC * P], f32r, kind="ExternalInput").ap()
    qkvw_in = nc.dram_tensor("qkv_wT", [NL, D, 3 * D], f32r, kind="ExternalInput").ap()
    qkvb_in = nc.dram_tensor("qkv_bt", [P, NL, 12], f32, kind="ExternalInput").ap()
    aow_in = nc.dram_tensor("ao_wT", [NL, D, D], f32r, kind="ExternalInput").ap()
    aob_in = nc.dram_tensor("ao_bt", [P, NL, KC], f32, kind="ExternalInput").ap()
    f1w_in = nc.dram_tensor("ff1_wT", [NL, D, 4 * D], f32r, kind="ExternalInput").ap()
    f1b_in = nc.dram_tensor("ff1_bt", [P, NL, 16], f32, kind="ExternalInput").ap()
    f2w_in = nc.dram_tensor("ff2_wT", [NL, 4 * D, D], f32r, kind="ExternalInput").ap()
    f2b_in = nc.dram_tensor("ff2_bt", [P, NL, KC], f32, kind="ExternalInput").ap()
    l1s_in = nc.dram_tensor("ln1_srow", [NL * KC * P], f32r, kind="ExternalInput").ap()
    l1b_in = nc.dram_tensor("ln1_bt", [P, NL, KC], f32, kind="ExternalInput").ap()
    l2s_in = nc.dram_tensor("ln2_srow", [NL * KC * P], f32r, kind="ExternalInput").ap()
    l2b_in = nc.dram_tensor("ln2_bt", [P, NL, KC], f32, kind="ExternalInput").ap()
    outw_in = nc.dram_tensor("out_wT", [D, V], f32r, kind="ExternalInput").ap()
    logits_out = nc.dram_tensor("logits", [TL, V], f32, kind="ExternalOutput").ap()

    with tile.TileContext(nc) as tc:
        with (
            tc.tile_pool(name="persist", bufs=1) as pp,
            tc.tile_pool(name="dram", bufs=2, space="DRAM") as dramp,
        ):
            ident = pp.tile([P, P], f32, tag="ident")
            nc.sync.dma_start(ident[:], ident_in[:])
            ones_f = pp.tile([P, 1], f32, tag="ones_f")
            nc.vector.memset(ones_f[:], 1.0)
            eps_t = pp.tile([P, 1], f32, tag="eps_t")
            nc.vector.memset(eps_t[:], 1e-5)
            ones_col = pp.tile([P, 1], f32r, tag="ones_col")
            nc.vector.tensor_copy(ones_col[:], ones_f[:])
            sel = pp.tile([P, P], f32r, tag="sel")  # row 64 = ones
            nc.vector.tensor_copy(sel[64:65, :], ones_f[64:65, :].to_broadcast([1, P]))
            onesrow = pp.tile([1, TL], f32r, tag="onesrow")
            nc.vector.tensor_copy(onesrow[:], ones_f[0:1, :].to_broadcast([1, TL]))

            entw = pp.tile([1, KC, P], f32r, tag="entw")
            nc.sync.dma_start(entw[:].rearrange("o c p -> o (c p)"), entw_in.unsqueeze(0))
            entb = pp.tile([1, KC, P], f32r, tag="entb")
            nc.sync.dma_start(entb[:].rearrange("o c p -> o (c p)"), entb_in.unsqueeze(0))
            l1s = pp.tile([1, NL, KC, P], f32r, tag="l1s")
            nc.sync.dma_start(l1s[:].rearrange("o n c p -> o (n c p)"), l1s_in.unsqueeze(0))
            l2s = pp.tile([1, NL, KC, P], f32r, tag="l2s")
            nc.sync.dma_start(l2s[:].rearrange("o n c p -> o (n c p)"), l2s_in.unsqueeze(0))
            qkvb = pp.tile([P, NL, 12], f32, tag="qkvb")
            nc.sync.dma_start(qkvb[:], qkvb_in[:])
            aob = pp.tile([P, NL, KC], f32, tag="aob")
            nc.sync.dma_start(aob[:], aob_in[:])
            f1b = pp.tile([P, NL, 16], f32, tag="f1b")
            nc.sync.dma_start(f1b[:], f1b_in[:])
            f2b = pp.tile([P, NL, KC], f32, tag="f2b")
            nc.sync.dma_start(f2b[:], f2b_in[:])
            l1b = pp.tile([P, NL, KC], f32, tag="l1b")
            nc.sync.dma_start(l1b[:], l1b_in[:])
            l2b = pp.tile([P, NL, KC], f32, tag="l2b")
            nc.sync.dma_start(l2b[:], l2b_in[:])

            h = pp.tile([P, KC, TL], f32r, tag="h")
            g = pp.tile([P, KC, TL], f32r, tag="g")
            qT = pp.tile([P, KC, TL], f32r, tag="qT")
            oT = pp.tile([P, KC, TL], f32r, tag="oT")
            recipt = pp.tile([P, 2, TL], f32r, tag="recipt")  # row 64 used

            def dump(name, tile_ap, shape):
                if not DEBUG_TAPS:
                    return
                out = nc.dram_tensor("tap_" + name, shape, f32, kind="ExternalOutput").ap()
                nc.sync.dma_start(out, tile_ap.bitcast(f32))

            for _rep in range(repeat):
                _embed_entropy(nc, tc, h, ident, entw, entb, onesrow,
                               emb_in, xidx_in, xent_in, emask_in)
                dump("h0", h[:], (P, KC, TL))
                for layer in range(NL):
                    _layernorm(nc, tc, h, g, l1s, l1b, ones_col, eps_t, layer)
                    if layer == 0:
                        dump("g1", g[:], (P, KC, TL))
                    _qkv_kv_exchange(nc, tc, dramp, g, qT, qkvw_in, qkvb,
                                     ident, layer)
                    if layer == 0:
                        dump("qT", qT[:], (P, KC, TL))
                    _attention(nc, tc, h, qT, oT, recipt, sel, aow_in, aob, layer)
                    if layer == 0:
                        dump("oT", oT[:], (P, KC, TL))
                        dump("h1", h[:], (P, KC, TL))
                    _layernorm(nc, tc, h, g, l2s, l2b, ones_col, eps_t, layer)
                    _ffn(nc, tc, h, g, f1w_in, f1b, f2w_in, f2b, layer)
                    if layer == 0:
                        dump("h1f", h[:], (P, KC, TL))
                _logits(nc, tc, h, outw_in, logits_out)

    nc.compile()
    return nc


def _embed_entropy(nc, tc, h, ident, entw, entb, onesrow,
                   emb_in, xidx_in, xent_in, emask_in):
    with (
        tc.tile_pool(name="h0sb", bufs=2) as h0sb,
        tc.tile_pool(name="h0ps", bufs=1, space="PSUM") as h0ps,
        tc.tile_pool(name="entsb", bufs=1) as entsb,
        tc.tile_pool(name="entdr", bufs=1, space="DRAM") as entdr,
    ):
        # --- entropy features (tiny) ---
        x15 = entsb.tile([P, 15], f32, tag="x15")
        nc.sync.dma_start(x15[:], _ap(xent_in, 0, [[8, P], [1, 15]]))
        ppair = list(x15[:].ap)[0]
        eqb = entsb.tile([P, 8, 8], f32, tag="eqb")
        rr = entsb.tile([P, 8, 8], f32, tag="rr")
        a_j = _ap(x15[:], 0, [ppair, [1, 8], [1, 8]])
        for u in range(8):
            a_u = _ap(x15[:], u, [ppair, [1, 8], [0, 8]])
            if u == 0:
                nc.vector.tensor_tensor(out=rr[:], in0=a_j, in1=a_u, op=EQ)
            else:
                nc.vector.tensor_tensor(out=eqb[:], in0=a_j, in1=a_u, op=EQ)
                nc.vector.tensor_tensor(out=rr[:], in0=rr[:], in1=eqb[:], op=ADD)
        lnr = entsb.tile([P, 8, 8], f32, tag="lnr")
        nc.scalar.activation(lnr[:], rr[:], LN_)
        bterm = entsb.tile([P, 8, 8], f32, tag="bterm")
        nc.vector.tensor_scalar(out=bterm[:], in0=lnr[:], scalar1=-INV_8LN2,
                                scalar2=3.0 / 8.0, op0=MUL, op1=ADD)
        m15 = entsb.tile([P, 15], f32, tag="m15")
        nc.vector.tensor_scalar(out=m15[:], in0=x15[:], scalar1=255.5,
                                scalar2=None, op0=LT)
        m_j = _ap(m15[:], 0, [list(m15[:].ap)[0], [1, 8], [1, 8]])
        nc.vector.tensor_tensor(out=bterm[:], in0=bterm[:], in1=m_j, op=MUL)
        ent8 = entsb.tile([P, 8], f32, tag="ent8")
        nc.vector.tensor_reduce(ent8[:], bterm[:], axis=mybir.AxisListType.X, op=ADD)
        emask = entsb.tile([P, 8], f32, tag="emask")
        nc.sync.dma_start(emask[:], emask_in.rearrange("(p a) -> p a", a=8))
        nc.vector.tensor_tensor(out=ent8[:], in0=ent8[:], in1=emask[:], op=MUL)
        entrow_f = entsb.tile([1, TL], f32, tag="entrow_f")
        ent_dram = entdr.tile([P, 8], f32, tag="ent_dram")
        nc.sync.dma_start(ent_dram[:], ent8[:])
        nc.sync.dma_start(entrow_f[:],
                          ent_dram[:].rearrange("p a -> (p a)").unsqueeze(0))
        entrow = entsb.tile([1, TL], f32r, tag="entrow")
        nc.vector.tensor_copy(entrow[:], entrow_f[:])
        if DEBUG_TAPS:
            tapent = nc.dram_tensor("tap_ent", [1, TL], f32, kind="ExternalOutput").ap()
            nc.sync.dma_start(tapent, entrow_f[:])
            tape8 = nc.dram_tensor("tap_ent8", [P, 8], f32, kind="ExternalOutput").ap()
            nc.sync.dma_start(tape8, ent8[:])
            tapbt = nc.dram_tensor("tap_bterm", [P, 8, 8], f32, kind="ExternalOutput").ap()
            nc.sync.dma_start(tapbt, bterm[:])
            tapr = nc.dram_tensor("tap_r", [P, 8, 8], f32, kind="ExternalOutput").ap()
            nc.sync.dma_start(tapr, rr[:])
            tapx15 = nc.dram_tensor("tap_x15", [P, 15], f32, kind="ExternalOutput").ap()
            nc.sync.dma_start(tapx15, x15[:])

        # --- embedding gather + transpose + ent outer products ---
        xi = h0sb.tile([P, 8], i32, tag="xi")
        nc.sync.dma_start(xi[:], xidx_in.rearrange("(gq p) -> p gq", p=P))
        pcs = [h0ps.tile([P, TL], f32, space="PSUM", tag=f"h0c{c}", name=f"h0c{c}") for c in range(KC)]
        for gq in range(8):
            tok = h0sb.tile([P, D], f32, tag="tok")
            nc.gpsimd.indirect_dma_start(
                out=tok[:], out_offset=None, in_=emb_in[:],
                in_offset=bass.IndirectOffsetOnAxis(ap=xi[:, gq:gq + 1], axis=0),
            )
            for c in range(KC):
                nc.tensor.matmul(
                    pcs[c][:, gq * P:(gq + 1) * P], tok[:, c * P:(c + 1) * P],
                    ident[:], is_transpose=True, start=True, stop=(gq == 7),
                    skip_group_check=True)
        for c in range(KC):
            with nc.allow_low_precision(reason="f32r rounding intentional"):
                nc.vector.tensor_copy(h[:, c], pcs[c][:])
        for c in range(KC):
            entps = h0ps.tile([P, TL], f32, space="PSUM", tag=f"h0c{c}",
                              name=f"entps{c}")
            for t in range(2):
                ts = slice(t * 512, (t + 1) * 512)
                nc.tensor.matmul(entps[:, ts], entw[0:1, c, :], entrow[0:1, ts],
                                 start=True, stop=False, skip_group_check=True)
                nc.tensor.matmul(entps[:, ts], entb[0:1, c, :], onesrow[0:1, ts],
                                 start=False, stop=True, skip_group_check=True)
            with nc.allow_low_precision(reason="f32r rounding intentional"):
                nc.vector.scalar_tensor_tensor(
                    out=h[:, c], in0=entps[:], scalar=1.0,
                    in1=h[:, c].bitcast(f32), op0=MUL, op1=ADD)


def _layernorm(nc, tc, h, g, srow, bt, ones_col, eps_t, layer):
    """g = LN(h) * s + b (feature-major; stats via PE ones-reduction)."""
    with (
        tc.tile_pool(name="lnps", bufs=2, space="PSUM") as lnps,
        tc.tile_pool(name="lnsq", bufs=1) as lnsq,
        tc.tile_pool(name="lnsb", bufs=2) as lnsb,
    ):
        sq = lnsq.tile([P, KC, TL], f32r, tag="sq")
        with nc.allow_low_precision(reason="f32r rounding intentional"):
            nc.vector.tensor_tensor(out=sq[:], in0=h[:].bitcast(f32),
                                    in1=h[:].bitcast(f32), op=MUL)
        stat = lnps.tile([1, 2, TL], f32, space="PSUM", tag="lnp")
        for c in range(KC):
            for t in range(2):
                ts = slice(t * 512, (t + 1) * 512)
                nc.tensor.matmul(stat[0:1, 0, ts], ones_col[:], h[:, c, ts],
                                 start=(c == 0), stop=(c == KC - 1),
                                 skip_group_check=True)
                nc.tensor.matmul(stat[0:1, 1, ts], ones_col[:], sq[:, c, ts],
                                 start=(c == 0), stop=(c == KC - 1),
                                 skip_group_check=True)
        mu = lnsb.tile([1, TL], f32, tag="mu")
        nc.vector.tensor_scalar(out=mu[:], in0=stat[0:1, 0, :], scalar1=1.0 / D,
                                scalar2=None, op0=MUL)
        musq = lnsb.tile([1, TL], f32, tag="musq")
        nc.vector.tensor_tensor(out=musq[:], in0=mu[:], in1=mu[:], op=MUL)
        var = lnsb.tile([1, TL], f32, tag="var")
        nc.vector.scalar_tensor_tensor(out=var[:], in0=stat[0:1, 1, :], scalar=1.0 / D,
                                       in1=musq[:], op0=MUL, op1=SUB)
        lnv = lnsb.tile([1, TL], f32, tag="lnv")
        nc.scalar.activation(lnv[:], var[:], LN_, bias=eps_t[0:1, :])
        rstd = lnsb.tile([1, TL], f32r, tag="rstd")
        nc.scalar.activation(rstd[:], lnv[:], EXP, scale=-0.5)
        nmr = lnsb.tile([1, TL], f32r, tag="nmr")
        with nc.allow_low_precision(reason="f32r rounding intentional"):
            nc.vector.scalar_tensor_tensor(out=nmr[:], in0=mu[:], scalar=-1.0,
                                           in1=rstd[:].bitcast(f32), op0=MUL, op1=MUL)
        for c in range(KC):
            bc = lnps.tile([P, 2, TL], f32, space="PSUM", tag="lnp")
            for t in range(2):
                ts = slice(t * 512, (t + 1) * 512)
                nc.tensor.matmul(bc[:, 0, ts], srow[0:1, layer, c, :], rstd[0:1, ts],
                                 start=True, stop=True)
                nc.tensor.matmul(bc[:, 1, ts], srow[0:1, layer, c, :], nmr[0:1, ts],
                                 start=True, stop=True)
            gtmp = lnsb.tile([P, TL], f32, tag="gtmp")
            nc.vector.scalar_tensor_tensor(out=gtmp[:], in0=bc[:, 0], scalar=1.0,
                                           in1=h[:, c].bitcast(f32), op0=MUL, op1=MUL)
            with nc.allow_low_precision(reason="f32r rounding intentional"):
                nc.vector.scalar_tensor_tensor(out=g[:, c], in0=bc[:, 1],
                                               scalar=bt[:, layer, c:c + 1],
                                               in1=gtmp[:], op0=ADD, op1=ADD)


def _qkv_kv_exchange(nc, tc, dramp, g, qT, qkvw_in, qkvb, ident, layer):
    bounce_in = dramp.tile([2, D, TL], f32r, tag="cc_in")
    bounce_out = dramp.tile([2, 2, D, TL], f32r, tag="cc_out")
    with (
        tc.tile_pool(name="qkvw", bufs=2) as wpool,
        tc.tile_pool(name="qkvps", bufs=3, space="PSUM") as qps,
        tc.tile_pool(name="vtps", bufs=2, space="PSUM") as vtps,
        tc.tile_pool(name="qkvsb", bufs=1) as qsb,
        tc.tile_pool(name="vtmp", bufs=1) as vtmp,
    ):
        qw_all = wpool.tile([P, KC, 3 * D], f32r, tag="qw")
        nc.sync.dma_start(qw_all[:],
                          qkvw_in[layer].rearrange("(c p) f -> p c f", p=P))
        kTl = qsb.tile([P, KC, TL], f32r, tag="kTl")
        vTl = qsb.tile([P, KC, TL], f32, tag="vTl")
        for j in [4, 5, 6, 7, 8, 9, 10, 11, 0, 1, 2, 3]:
            ps = qps.tile([P, TL], f32, space="PSUM", tag="qkvp")
            for c in range(KC):
                for t in range(2):
                    ts = slice(t * 512, (t + 1) * 512)
                    nc.tensor.matmul(ps[:, ts], qw_all[:, c, j * P:(j + 1) * P],
                                     g[:, c, ts], start=(c == 0), stop=(c == KC - 1))
            if j < 4:
                dest = qT[:, j]
            elif j < 8:
                dest = kTl[:, j - 4]
            else:
                dest = vTl[:, j - 8]
            with nc.allow_low_precision(reason="f32r rounding intentional"):
                nc.vector.tensor_scalar(out=dest, in0=ps[:],
                                        scalar1=qkvb[:, layer, j:j + 1],
                                        scalar2=None, op0=ADD)
            if 4 <= j < 8:
                nc.sync.dma_start(bounce_in[0, (j - 4) * P:(j - 3) * P, :],
                                  kTl[:, j - 4])
        v_tm_dram = bounce_in[1].rearrange("d t -> (d t)").rearrange(
            "(t d) -> t d", d=D)
        vtm = vtmp.tile([P, 8, D], f32r, tag="vtm")
        for gq in range(8):
            pv = vtps.tile([P, D], f32, space="PSUM", tag="vtp")
            for c in range(KC):
                nc.tensor.matmul(pv[:, c * P:(c + 1) * P],
                                 vTl[:, c, gq * P:(gq + 1) * P], ident[:],
                                 is_transpose=True, start=True, stop=True)
            with nc.allow_low_precision(reason="f32r rounding intentional"):
                nc.vector.tensor_copy(vtm[:, gq], pv[:])
        nc.sync.dma_start(v_tm_dram.rearrange("(gq p) d -> p gq d", p=P), vtm[:])
        if SIM_MODE or NO_COLLECTIVE:
            for half in range(2):
                nc.sync.dma_start(bounce_out[half], bounce_in[:])
        else:
            nc.gpsimd.collective_compute(
                "AllGather", mybir.AluOpType.bypass,
                replica_groups=[[0, 1], [2, 3], [4, 5], [6, 7]],
                ins=[bounce_in.opt()], outs=[bounce_out.opt()],
            )
    _qkv_kv_exchange._bounce_out = bounce_out


def _attention(nc, tc, h, qT, oT, recipt, sel, aow_in, aob, layer):
    bounce_out = _qkv_kv_exchange._bounce_out
    with (
        tc.tile_pool(name="attkv", bufs=2) as kvp,
        tc.tile_pool(name="attex", bufs=3) as exp_p,
        tc.tile_pool(name="attops", bufs=1, space="PSUM") as opsp,
        tc.tile_pool(name="attscs", bufs=2, space="PSUM") as scps,
        tc.tile_pool(name="attsb", bufs=2) as attsb,
    ):
        for pair in range(H // 2):
            c = pair
            h1 = 2 * pair
            kTp = kvp.tile([P, L], f32r, tag="kTp")
            nc.sync.dma_start(
                kTp[:].rearrange("p (hf t) -> p hf t", hf=2),
                bounce_out[:, 0, c * P:(c + 1) * P, :].rearrange("hf p t -> p hf t"))
            vaug = kvp.tile([P, MC, 2, HD + 1], f32r, tag="vaug")
            for half in range(2):
                vsrc = bounce_out[half, 1].rearrange("d t -> (d t)").rearrange(
                    "(m p hh d) -> p m hh d", p=P, hh=H, d=HD)
                for u in range(2):
                    nc.sync.dma_start(
                        vaug[:, 8 * half:8 * half + 8, u, 0:HD],
                        vsrc[:, :, h1 + u, :])
            onesf2 = kvp.tile([P, 1], f32, tag="onesf2")
            nc.vector.memset(onesf2[:], 1.0)
            nc.vector.tensor_copy(
                vaug[:, :, :, HD:HD + 1].rearrange("p m hh one -> p (m hh one)"),
                onesf2[:].to_broadcast([P, MC * 2]))
            o_ps1 = opsp.tile([HD + 1, TL], f32, space="PSUM", tag="ops1")
            o_ps2 = opsp.tile([HD + 1, TL], f32, space="PSUM", tag="ops2")
            for m in range(MC):
                for t in range(2):
                    ts = slice(t * 512, (t + 1) * 512)
                    s_ps = scps.tile([P, 2, 512], f32, space="PSUM", tag="sps")
                    exps = exp_p.tile([P, 2, 512], f32r, tag="exps")
                    nc.tensor.matmul(s_ps[:, 0], kTp[0:HD, m * P:(m + 1) * P],
                                     qT[0:HD, c, ts], start=True, stop=True)
                    nc.tensor.matmul(s_ps[:, 1], kTp[HD:P, m * P:(m + 1) * P],
                                     qT[HD:P, c, ts], start=True, stop=True)
                    nc.scalar.activation(exps[:], s_ps[:], EXP, scale=ATT_SCALE)
                    nc.tensor.matmul(o_ps1[:, ts], vaug[:, m, 0], exps[:, 0],
                                     start=(m == 0), stop=(m == MC - 1))
                    nc.tensor.matmul(o_ps2[:, ts], vaug[:, m, 1], exps[:, 1],
                                     start=(m == 0), stop=(m == MC - 1))
            with nc.allow_low_precision(reason="f32r rounding intentional"):
                nc.vector.reciprocal(recipt[64:65, 0], o_ps1[64:65, :])
                nc.vector.reciprocal(recipt[64:65, 1], o_ps2[64:65, :])
            for t in range(2):
                ts = slice(t * 512, (t + 1) * 512)
                rbt = scps.tile([P, 2, 512], f32, space="PSUM", tag="sps")
                nc.tensor.matmul(rbt[0:HD, 0], sel[64:65, 0:HD], recipt[64:65, 0, ts],
                                 start=True, stop=True)
                nc.tensor.matmul(rbt[0:HD, 1], sel[64:65, 0:HD], recipt[64:65, 1, ts],
                                 start=True, stop=True)
                rbs = attsb.tile([P, 2, 512], f32, tag="rbs")
                nc.vector.tensor_copy(rbs[0:HD], rbt[0:HD])
                with nc.allow_low_precision(reason="f32r rounding intentional"):
                    nc.vector.scalar_tensor_tensor(
                        out=oT[0:HD, c, ts], in0=o_ps1[0:HD, ts], scalar=1.0,
                        in1=rbs[0:HD, 0], op0=MUL, op1=MUL)
                    nc.vector.scalar_tensor_tensor(
                        out=oT[HD:P, c, ts], in0=o_ps2[0:HD, ts], scalar=1.0,
                        in1=rbs[0:HD, 1], op0=MUL, op1=MUL)
    with (
        tc.tile_pool(name="aow", bufs=1) as aowp,
        tc.tile_pool(name="aops", bufs=2, space="PSUM") as aops,
    ):
        aw_all = aowp.tile([P, KC, D], f32r, tag="aw")
        nc.sync.dma_start(aw_all[:], aow_in[layer].rearrange("(c p) f -> p c f", p=P))
        for j in range(KC):
            ps = aops.tile([P, TL], f32, space="PSUM", tag="aop")
            for c in range(KC):
                for t in range(2):
                    ts = slice(t * 512, (t + 1) * 512)
                    nc.tensor.matmul(ps[:, ts], aw_all[:, c, j * P:(j + 1) * P],
                                     oT[:, c, ts], start=(c == 0), stop=(c == KC - 1))
            with nc.allow_low_precision(reason="f32r rounding intentional"):
                nc.vector.scalar_tensor_tensor(
                    out=h[:, j], in0=ps[:], scalar=aob[:, layer, j:j + 1],
                    in1=h[:, j].bitcast(f32), op0=ADD, op1=ADD)


def _ffn(nc, tc, h, g, f1w_in, f1b, f2w_in, f2b, layer):
    with (
        tc.tile_pool(name="f1w", bufs=1) as f1wp,
        tc.tile_pool(name="f2w", bufs=1) as f2wp,
        tc.tile_pool(name="zp", bufs=1) as zp,
        tc.tile_pool(name="btp", bufs=2) as btp,
        tc.tile_pool(name="f1ps", bufs=2, space="PSUM") as f1ps,
        tc.tile_pool(name="f2ps", bufs=1, space="PSUM") as f2ps,
    ):
        f1w_all = f1wp.tile([P, KC, 4 * D], f32r, tag="f1w")
        nc.sync.dma_start(f1w_all[:], f1w_in[layer].rearrange("(c p) f -> p c f", p=P))
        f2w_all = f2wp.tile([P, 16, D], f32r, tag="f2w")
        nc.sync.dma_start(f2w_all[:], f2w_in[layer].rearrange("(k p) f -> p k f", p=P))
        for th in range(2):
            ths = slice(th * 512, (th + 1) * 512)
            z = zp.tile([P, 16, 512], f32r, tag="z")
            for jp in range(8):
                ps = f1ps.tile([P, 2, 512], f32, space="PSUM", tag="f1p")
                for u in range(2):
                    j = 2 * jp + u
                    for c in range(KC):
                        nc.tensor.matmul(
                            ps[:, u], f1w_all[:, c, j * P:(j + 1) * P],
                            g[:, c, ths], start=(c == 0), stop=(c == KC - 1))
                bs = btp.tile([P, 2, 512], f32, tag="btmp")
                nc.vector.tensor_scalar(out=bs[:, 0], in0=ps[:, 0],
                                        scalar1=f1b[:, layer, 2 * jp:2 * jp + 1],
                                        scalar2=None, op0=ADD)
                nc.vector.tensor_scalar(out=bs[:, 1], in0=ps[:, 1],
                                        scalar1=f1b[:, layer, 2 * jp + 1:2 * jp + 2],
                                        scalar2=None, op0=ADD)
                with nc.allow_low_precision(reason="f32r rounding intentional"):
                    nc.scalar.activation(
                        z[:, 2 * jp:2 * jp + 2, :].rearrange("p a b -> p (a b)"),
                        bs[:].rearrange("p a b -> p (a b)"), GELU)
            pso = [f2ps.tile([P, 512], f32, space="PSUM", tag=f"f2p{i}", name=f"f2p{i}")
                   for i in range(KC)]
            for k in range(16):
                for i in range(KC):
                    nc.tensor.matmul(pso[i][:], f2w_all[:, k, i * P:(i + 1) * P],
                                     z[:, k, :], start=(k == 0), stop=(k == 15))
            for i in range(KC):
                with nc.allow_low_precision(reason="f32r rounding intentional"):
                    nc.vector.scalar_tensor_tensor(
                        out=h[:, i, ths], in0=pso[i][:], scalar=f2b[:, layer, i:i + 1],
                        in1=h[:, i, ths].bitcast(f32), op0=ADD, op1=ADD)


def _logits(nc, tc, h, outw_in, logits_out):
    with (
        tc.tile_pool(name="lgsb", bufs=3) as lgsb,
        tc.tile_pool(name="lgw", bufs=1) as lgw,
        tc.tile_pool(name="lgps", bufs=3, space="PSUM") as lgps,
    ):
        oww = lgw.tile([P, KC, V], f32r, tag="oww")
        nc.sync.dma_start(oww[:], outw_in.rearrange("(c p) v -> p c v", p=P))
        for gq in range(8):
            ps = lgps.tile([P, V], f32, space="PSUM", tag="lgp")
            for c in range(KC):
                nc.tensor.matmul(ps[:], h[:, c, gq * P:(gq + 1) * P],
                                 oww[:, c], start=(c == 0), stop=(c == KC - 1))
            lg = lgsb.tile([P, V], f32, tag="lg")
            nc.vector.tensor_copy(lg[:], ps[:])
            nc.sync.dma_start(logits_out[gq * P:(gq + 1) * P, :], lg[:])


def _host_prep(inputs):
    emb = np.ascontiguousarray(inputs["emb"], dtype=np.float32)
    x = np.asarray(inputs["x"])
    ident = np.eye(P, dtype=np.float32)
    entw_row = np.ascontiguousarray(np.asarray(inputs["ent_w"])[:, 0], np.float32)
    entb_row = np.ascontiguousarray(np.asarray(inputs["ent_b"]), np.float32)
    qkv_wT = np.ascontiguousarray(np.transpose(inputs["qkv_w"], (0, 2, 1)), np.float32)
    qkv_bt = np.ascontiguousarray(np.asarray(inputs["qkv_b"]).reshape(NL, 12, P).transpose(2, 0, 1), np.float32)
    ao_wT = np.ascontiguousarray(np.transpose(inputs["ao_w"], (0, 2, 1)), np.float32)
    ao_bt = np.ascontiguousarray(np.asarray(inputs["ao_b"]).reshape(NL, KC, P).transpose(2, 0, 1), np.float32)
    ff1_wT = np.ascontiguousarray(np.transpose(inputs["ff1_w"], (0, 2, 1)), np.float32)
    ff1_bt = np.ascontiguousarray(np.asarray(inputs["ff1_b"]).reshape(NL, 16, P).transpose(2, 0, 1), np.float32)
    ff2_wT = np.ascontiguousarray(np.transpose(inputs["ff2_w"], (0, 2, 1)), np.float32)
    ff2_bt = np.ascontiguousarray(np.asarray(inputs["ff2_b"]).reshape(NL, KC, P).transpose(2, 0, 1), np.float32)
    ln1_srow = np.ascontiguousarray(np.asarray(inputs["ln1_s"]).reshape(NL * KC * P), np.float32)
    ln1_bt = np.ascontiguousarray(np.asarray(inputs["ln1_b"]).reshape(NL, KC, P).transpose(2, 0, 1), np.float32)
    ln2_srow = np.ascontiguousarray(np.asarray(inputs["ln2_s"]).reshape(NL * KC * P), np.float32)
    ln2_bt = np.ascontiguousarray(np.asarray(inputs["ln2_b"]).reshape(NL, KC, P).transpose(2, 0, 1), np.float32)
    out_wT = np.ascontiguousarray(np.asarray(inputs["out_w"]).T, np.float32)

    shared = dict(emb=emb, ident=ident, entw_row=entw_row, entb_row=entb_row,
                  qkv_wT=qkv_wT, qkv_bt=qkv_bt, ao_wT=ao_wT, ao_bt=ao_bt,
                  ff1_wT=ff1_wT, ff1_bt=ff1_bt, ff2_wT=ff2_wT, ff2_bt=ff2_bt,
                  ln1_srow=ln1_srow, ln1_bt=ln1_bt, ln2_srow=ln2_srow,
                  ln2_bt=ln2_bt, out_wT=out_wT)
    in_maps = []
    for core in range(NCORE):
        b, s = divmod(core, 2)
        t0 = s * TL
        xb = np.asarray(x[b], dtype=np.int64)
        x_idx = xb[t0:t0 + TL].astype(np.int32)
        xpad = np.concatenate([xb, np.zeros(8, np.int64)])
        x_ent = xpad[t0:t0 + 1032].astype(np.float32)
        pos = np.arange(t0, t0 + TL)
        ent_mask = (pos <= L - W).astype(np.float32)
        in_maps.append(dict(shared, x_idx=x_idx, x_ent=x_ent, ent_mask=ent_mask))
    return in_maps


def kernel(**inputs) -> np.ndarray:
    nc = build_program(1)
    in_maps = _host_prep(inputs)
    res = run_bass_kernel_spmd(nc, in_maps, list(range(NCORE)))
    logits = np.empty((B, L, V), np.float32)
    for core in range(NCORE):
        b, s = divmod(core, 2)
        logits[b, s * TL:(s + 1) * TL, :] = res.results[core]["logits"]
    return logits



# revision 13
# speedup vs baseline: 6.4704x; 6.4704x over previous
"""EntropyByteLatentTransformer Trainium2 kernel (8 NeuronCores), v2.

Sharding: batch b = core//2, sequence half s = core%2 (1024 tokens/core).
Per layer each core computes q/k/v for its tokens; k (feature-major) and v
(token-major) are exchanged within the core pair via two AllGathers (k first,
launched mid-QKV so the link transfer overlaps the v/q matmuls) so every core
attends over the full 2048-key sequence for its 1024 queries.

v2 vs v1:
- GEMM operands in bf16 (weights pre-cast on host; LN/attention activations
  written as bf16). Residual stream h stays f32r; psum accumulation f32.
- PSUM evacuations moved to the scalar (Act) engine as Copy/Gelu activations
  with the bias folded in ([P,1] bias AP), freeing the vector engine.
- v computed directly token-major (g-stationary matmuls) - no PE transpose.
- v bias applied after attention (exp@(v+b) = exp@v + b*denom).
- Split k/v collectives in bf16 (half the link bytes), launched as soon as
  their operands are staged; attention consumes per-pair so pair 0 starts
  after the k gather + its own DMA only.
- Weight prefetch: qkv weights for layer l+1 issued during FFN l, ff1/ff2
  weights issued at attention start, double-buffered pools.
- LayerNorm split into independent 512-token halves (shorter serial chain,
  overlaps the neighbouring phases).
- Attention m-loop software-pipelined: scores m+1 issued before attn@v m so
  the PE never waits on the Act-engine exp.
"""
import numpy as np
from functools import lru_cache

import ml_dtypes

import concourse.bass as bass
import concourse.tile as tile
import concourse.mybir as mybir
from concourse import bacc
from concourse.bass_utils import run_bass_kernel_spmd

B, L, D, H, NL, V, W = 4, 2048, 512, 8, 4, 258, 8
HD = D // H          # 64
P = 128
TL = L // 2          # 1024 tokens per core
KC = D // P          # 4 feature chunks
MC = L // P          # 16 key chunks (full seq)
NCORE = 8

f32 = mybir.dt.float32
f32r = mybir.dt.float32r
bf16 = mybir.dt.bfloat16
i32 = mybir.dt.int32
EXP = mybir.ActivationFunctionType.Exp
LN_ = mybir.ActivationFunctionType.Ln
GELU = mybir.ActivationFunctionType.Gelu
COPY = mybir.ActivationFunctionType.Copy
IDENT = mybir.ActivationFunctionType.Identity
ADD = mybir.AluOpType.add
SUB = mybir.AluOpType.subtract
MUL = mybir.AluOpType.mult
EQ = mybir.AluOpType.is_equal
LT = mybir.AluOpType.is_lt

SIM_MODE = False  # replace collectives with local copies (single-core TimelineSim)
NO_COLLECTIVE = False  # 8-core run but collectives replaced by local DMA (ablation)


def _lnexp_set_id():
    from concourse.hw_specs import get_activation_tables
    return list(get_activation_tables("gen3")).index("natural_log_exp_and_others")

INV_8LN2 = float(1.0 / (8.0 * np.log(2.0)))
ATT_SCALE = float(1.0 / np.sqrt(HD))


def _ap(base_ap, offset_elems, pairs):
    """Raw AP over the same tensor with explicit [step, count] pairs."""
    return bass.AP(base_ap.tensor, base_ap.offset + offset_elems, pairs)


@lru_cache(maxsize=None)
def build_program(repeat=1):
    nc = bacc.Bacc("TRN2", target_bir_lowering=False, debug=False,
                   num_devices=1 if SIM_MODE else NCORE)

    emb_in = nc.dram_tensor("emb", [V, D], f32, kind="ExternalInput").ap()
    ident_in = nc.dram_tensor("ident", [P, P], f32, kind="ExternalInput").ap()
    xidx_in = nc.dram_tensor("x_idx", [TL], i32, kind="ExternalInput").ap()
    xent_in = nc.dram_tensor("x_ent", [1032], f32, kind="ExternalInput").ap()
    emask_in = nc.dram_tensor("ent_mask", [TL], f32, kind="ExternalInput").ap()
    entw_in = nc.dram_tensor("entw_row", [KC * P], f32r, kind="ExternalInput").ap()
    entb_in = nc.dram_tensor("entb_row", [KC * P], f32r, kind="ExternalInput").ap()
    qkvw_in = nc.dram_tensor("qkv_wT", [NL, D, 3 * D], bf16, kind="ExternalInput").ap()
    qkvb_in = nc.dram_tensor("qkv_bt", [P, NL, 12], f32, kind="ExternalInput").ap()
    aow_in = nc.dram_tensor("ao_wT", [NL, D, D], bf16, kind="ExternalInput").ap()
    aob_in = nc.dram_tensor("ao_bt", [P, NL, KC], f32, kind="ExternalInput").ap()
    f1w_in = nc.dram_tensor("ff1_wT", [NL, D, 4 * D], bf16, kind="ExternalInput").ap()
    f1b_in = nc.dram_tensor("ff1_bt", [P, NL, 16], f32, kind="ExternalInput").ap()
    f2w_in = nc.dram_tensor("ff2_wT", [NL, 4 * D, D], bf16, kind="ExternalInput").ap()
    f2b_in = nc.dram_tensor("ff2_bt", [P, NL, KC], f32, kind="ExternalInput").ap()
    l1s_in = nc.dram_tensor("ln1_srow", [NL, KC * P], bf16, kind="ExternalInput").ap()
    l1b_in = nc.dram_tensor("ln1_bt", [P, NL, KC], f32, kind="ExternalInput").ap()
    l2s_in = nc.dram_tensor("ln2_srow", [NL, KC * P], bf16, kind="ExternalInput").ap()
    l2b_in = nc.dram_tensor("ln2_bt", [P, NL, KC], f32, kind="ExternalInput").ap()
    outw_in = nc.dram_tensor("out_wT", [D, V], f32r, kind="ExternalInput").ap()
    logits_out = nc.dram_tensor("logits", [TL, V], f32, kind="ExternalOutput").ap()

    with tile.TileContext(nc) as tc:
        with (
            tc.tile_pool(name="persist", bufs=1) as pp,
            tc.tile_pool(name="wq", bufs=2) as wqp,
            tc.tile_pool(name="wao", bufs=1) as waop,
            tc.tile_pool(name="wf1", bufs=2) as wf1p,
            tc.tile_pool(name="wf2", bufs=2) as wf2p,
            tc.tile_pool(name="lnrow", bufs=2) as lnrp,
            tc.tile_pool(name="dram", bufs=2, space="DRAM") as dramp,
        ):
            ident = pp.tile([P, P], f32, tag="ident")
            nc.sync.dma_start(ident[:], ident_in[:])
            ones_f = pp.tile([P, 1], f32, tag="ones_f")
            nc.vector.memset(ones_f[:], 1.0)
            eps_t = pp.tile([P, 1], f32, tag="eps_t")
            nc.vector.memset(eps_t[:], 1e-5)
            ones_col = pp.tile([P, 1], f32r, tag="ones_col")
            nc.vector.tensor_copy(ones_col[:], ones_f[:])
            sel = pp.tile([P, HD], bf16, tag="sel")  # row 64 = ones
            nc.vector.tensor_copy(sel[64:65, :], ones_f[64:65, :].to_broadcast([1, HD]))
            onesrow = pp.tile([1, TL], f32r, tag="onesrow")
            nc.vector.tensor_copy(onesrow[:], ones_f[0:1, :].to_broadcast([1, TL]))

            entw = pp.tile([1, KC, P], f32r, tag="entw")
            nc.sync.dma_start(entw[:].rearrange("o c p -> o (c p)"), entw_in.unsqueeze(0))
            entb = pp.tile([1, KC, P], f32r, tag="entb")
            nc.sync.dma_start(entb[:].rearrange("o c p -> o (c p)"), entb_in.unsqueeze(0))
            qkvb = pp.tile([P, NL, 12], f32, tag="qkvb")
            nc.sync.dma_start(qkvb[:], qkvb_in[:])
            aob = pp.tile([P, NL, KC], f32, tag="aob")
            nc.sync.dma_start(aob[:], aob_in[:])
            f1b = pp.tile([P, NL, 16], f32, tag="f1b")
            nc.sync.dma_start(f1b[:], f1b_in[:])
            f2b = pp.tile([P, NL, KC], f32, tag="f2b")
            nc.sync.dma_start(f2b[:], f2b_in[:])
            l1b = pp.tile([P, NL, KC], f32, tag="l1b")
            nc.sync.dma_start(l1b[:], l1b_in[:])
            l2b = pp.tile([P, NL, KC], f32, tag="l2b")
            nc.sync.dma_start(l2b[:], l2b_in[:])

            h = pp.tile([P, KC, TL], f32r, tag="h")
            g = pp.tile([P, KC, TL], bf16, tag="g")      # LN out; also attn out
            qT = pp.tile([P, KC, TL], bf16, tag="qT")
            kT_all = pp.tile([P, KC, L], bf16, tag="kT_all")
            recipt = pp.tile([P, 2, TL], bf16, tag="recipt")  # row 64 used

            # prefetch layer-0 qkv weights during the embedding phase
            qw = wqp.tile([P, KC, 3 * D], bf16, tag="qw")
            nc.sync.dma_start(qw[:], qkvw_in[0].rearrange("(c p) f -> p c f", p=P))

            for _rep in range(repeat):
                _embed_entropy(nc, tc, h, ident, entw, entb, onesrow,
                               emb_in, xidx_in, xent_in, emask_in)
                for layer in range(NL):
                    if _rep > 0 or layer > 0:
                        qw = wqp.tile([P, KC, 3 * D], bf16, tag="qw")
                        nc.sync.dma_start(
                            qw[:], qkvw_in[layer].rearrange("(c p) f -> p c f", p=P))
                    l1s = lnrp.tile([1, KC, P], bf16, tag="l1s")
                    nc.sync.dma_start(l1s[:].rearrange("o c p -> o (c p)"),
                                      l1s_in[layer].unsqueeze(0))
                    l2s = lnrp.tile([1, KC, P], bf16, tag="l2s")
                    nc.sync.dma_start(l2s[:].rearrange("o c p -> o (c p)"),
                                      l2s_in[layer].unsqueeze(0))
                    _layernorm(nc, tc, h, g, l1s, l1b, ones_col, eps_t, layer)
                    bk_out, bv_out = _qkv_kv_exchange(
                        nc, tc, dramp, g, qT, kT_all, qw, qkvb, layer)
                    aw = waop.tile([P, KC, D], bf16, tag="aw")
                    nc.sync.dma_start(aw[:], aow_in[layer].rearrange("(c p) f -> p c f", p=P))
                    f1w = wf1p.tile([P, KC, 4 * D], bf16, tag="f1w")
                    nc.sync.dma_start(f1w[:], f1w_in[layer].rearrange("(c p) f -> p c f", p=P))
                    f2w = wf2p.tile([P, 16, D], bf16, tag="f2w")
                    nc.sync.dma_start(f2w[:], f2w_in[layer].rearrange("(k p) f -> p k f", p=P))
                    _attention(nc, tc, h, qT, g, kT_all, recipt, sel,
                               bk_out, bv_out, aw, qkvb, aob, layer)
                    _layernorm(nc, tc, h, g, l2s, l2b, ones_col, eps_t, layer)
                    _ffn(nc, tc, h, g, f1w, f1b, f2w, f2b, layer)
                _logits(nc, tc, h, outw_in, logits_out)

    nc.compile()
    return nc


def _embed_entropy(nc, tc, h, ident, entw, entb, onesrow,
                   emb_in, xidx_in, xent_in, emask_in):
    with (
        tc.tile_pool(name="h0sb", bufs=3) as h0sb,
        tc.tile_pool(name="h0ps", bufs=1, space="PSUM") as h0ps,
        tc.tile_pool(name="entsb", bufs=1) as entsb,
        tc.tile_pool(name="entdr", bufs=1, space="DRAM") as entdr,
    ):
        # --- entropy features (tiny) ---
        x15 = entsb.tile([P, 15], f32, tag="x15")
        nc.sync.dma_start(x15[:], _ap(xent_in, 0, [[8, P], [1, 15]]))
        ppair = list(x15[:].ap)[0]
        eqb = entsb.tile([P, 8, 8], f32, tag="eqb")
        rr = entsb.tile([P, 8, 8], f32, tag="rr")
        a_j = _ap(x15[:], 0, [ppair, [1, 8], [1, 8]])
        for u in range(8):
            a_u = _ap(x15[:], u, [ppair, [1, 8], [0, 8]])
            if u == 0:
                nc.vector.tensor_tensor(out=rr[:], in0=a_j, in1=a_u, op=EQ)
            else:
                nc.vector.tensor_tensor(out=eqb[:], in0=a_j, in1=a_u, op=EQ)
                nc.vector.tensor_tensor(out=rr[:], in0=rr[:], in1=eqb[:], op=ADD)
        lnr = entsb.tile([P, 8, 8], f32, tag="lnr")
        nc.scalar.activation(lnr[:], rr[:], LN_)
        bterm = entsb.tile([P, 8, 8], f32, tag="bterm")
        nc.vector.tensor_scalar(out=bterm[:], in0=lnr[:], scalar1=-INV_8LN2,
                                scalar2=3.0 / 8.0, op0=MUL, op1=ADD)
        m15 = entsb.tile([P, 15], f32, tag="m15")
        nc.vector.tensor_scalar(out=m15[:], in0=x15[:], scalar1=255.5,
                                scalar2=None, op0=LT)
        m_j = _ap(m15[:], 0, [list(m15[:].ap)[0], [1, 8], [1, 8]])
        nc.vector.tensor_tensor(out=bterm[:], in0=bterm[:], in1=m_j, op=MUL)
        ent8 = entsb.tile([P, 8], f32, tag="ent8")
        nc.vector.tensor_reduce(ent8[:], bterm[:], axis=mybir.AxisListType.X, op=ADD)
        emask = entsb.tile([P, 8], f32, tag="emask")
        nc.sync.dma_start(emask[:], emask_in.rearrange("(p a) -> p a", a=8))
        nc.vector.tensor_tensor(out=ent8[:], in0=ent8[:], in1=emask[:], op=MUL)
        entrow_f = entsb.tile([1, TL], f32, tag="entrow_f")
        ent_dram = entdr.tile([P, 8], f32, tag="ent_dram")
        nc.sync.dma_start(ent_dram[:], ent8[:])
        nc.sync.dma_start(entrow_f[:],
                          ent_dram[:].rearrange("p a -> (p a)").unsqueeze(0))
        entrow = entsb.tile([1, TL], f32r, tag="entrow")
        nc.vector.tensor_copy(entrow[:], entrow_f[:])

        # --- embedding gather + transpose + ent outer products ---
        xi = h0sb.tile([P, 8], i32, tag="xi")
        nc.sync.dma_start(xi[:], xidx_in.rearrange("(gq p) -> p gq", p=P))
        pcs = [h0ps.tile([P, TL], f32, space="PSUM", tag=f"h0c{c}", name=f"h0c{c}") for c in range(KC)]
        for gq in range(8):
            tok = h0sb.tile([P, D], f32, tag="tok")
            nc.gpsimd.indirect_dma_start(
                out=tok[:], out_offset=None, in_=emb_in[:],
                in_offset=bass.IndirectOffsetOnAxis(ap=xi[:, gq:gq + 1], axis=0),
            )
            for c in range(KC):
                nc.tensor.matmul(
                    pcs[c][:, gq * P:(gq + 1) * P], tok[:, c * P:(c + 1) * P],
                    ident[:], is_transpose=True, start=True, stop=(gq == 7),
                    skip_group_check=True)
        for c in range(KC):
            with nc.allow_low_precision(reason="f32r rounding intentional"):
                nc.vector.tensor_copy(h[:, c], pcs[c][:])
        for c in range(KC):
            entps = h0ps.tile([P, TL], f32, space="PSUM", tag=f"h0c{c}",
                              name=f"entps{c}")
            for t in range(2):
                ts = slice(t * 512, (t + 1) * 512)
                nc.tensor.matmul(entps[:, ts], entw[0:1, c, :], entrow[0:1, ts],
                                 start=True, stop=False, skip_group_check=True)
                nc.tensor.matmul(entps[:, ts], entb[0:1, c, :], onesrow[0:1, ts],
                                 start=False, stop=True, skip_group_check=True)
            with nc.allow_low_precision(reason="f32r rounding intentional"):
                nc.vector.scalar_tensor_tensor(
                    out=h[:, c], in0=entps[:], scalar=1.0,
                    in1=h[:, c].bitcast(f32), op0=MUL, op1=ADD)


def _layernorm(nc, tc, h, g, srow, bt, ones_col, eps_t, layer):
    """g = LN(h) * s + b (feature-major; stats via PE ones-reduction).

    Processed as two independent 512-token halves so the serial
    stats->rstd->broadcast chain is half as long and halves pipeline.
    """
    with (
        tc.tile_pool(name="lnps", bufs=2, space="PSUM") as lnps,
        tc.tile_pool(name="lnbc", bufs=2, space="PSUM") as lnbc,
        tc.tile_pool(name="lnsq", bufs=2) as lnsq,
        tc.tile_pool(name="lnsb", bufs=2) as lnsb,
        tc.tile_pool(name="lngt", bufs=2) as lngt,
    ):
        for t in range(2):
            ts = slice(t * 512, (t + 1) * 512)
            stat = lnps.tile([1, 2, 512], f32, space="PSUM", tag="lnp")
            for c in range(KC):
                sq = lnsq.tile([P, 512], f32r, tag="sq")
                with nc.allow_low_precision(reason="f32r rounding intentional"):
                    nc.vector.tensor_tensor(out=sq[:], in0=h[:, c, ts].bitcast(f32),
                                            in1=h[:, c, ts].bitcast(f32), op=MUL)
                nc.tensor.matmul(stat[0:1, 0, :], ones_col[:], h[:, c, ts],
                                 start=(c == 0), stop=(c == KC - 1),
                                 skip_group_check=True)
                nc.tensor.matmul(stat[0:1, 1, :], ones_col[:], sq[:],
                                 start=(c == 0), stop=(c == KC - 1),
                                 skip_group_check=True)
            mu = lnsb.tile([1, 512], f32, tag="mu")
            nc.vector.tensor_scalar(out=mu[:], in0=stat[0:1, 0, :], scalar1=1.0 / D,
                                    scalar2=None, op0=MUL)
            musq = lnsb.tile([1, 512], f32, tag="musq")
            nc.vector.tensor_tensor(out=musq[:], in0=mu[:], in1=mu[:], op=MUL)
            var = lnsb.tile([1, 512], f32, tag="var")
            nc.vector.scalar_tensor_tensor(out=var[:], in0=stat[0:1, 1, :],
                                           scalar=1.0 / D, in1=musq[:],
                                           op0=MUL, op1=SUB)
            lnv = lnsb.tile([1, 512], f32, tag="lnv")
            nc.scalar.activation(lnv[:], var[:], LN_, bias=eps_t[0:1, :])
            rstd = lnsb.tile([1, 512], bf16, tag="rstd")
            nc.scalar.activation(rstd[:], lnv[:], EXP, scale=-0.5)
            nmr = lnsb.tile([1, 512], bf16, tag="nmr")
            with nc.allow_low_precision(reason="bf16 rounding intentional"):
                nc.vector.scalar_tensor_tensor(out=nmr[:], in0=mu[:], scalar=-1.0,
                                               in1=rstd[:], op0=MUL, op1=MUL)
            for c in range(KC):
                bc = lnbc.tile([P, 2, 512], f32, space="PSUM", tag="lnb")
                nc.tensor.matmul(bc[:, 0], srow[0:1, c, :], rstd[0:1, :],
                                 start=True, stop=True)
                nc.tensor.matmul(bc[:, 1], srow[0:1, c, :], nmr[0:1, :],
                                 start=True, stop=True)
                gtmp = lngt.tile([P, 512], f32, tag="gtmp")
                nc.vector.scalar_tensor_tensor(out=gtmp[:], in0=bc[:, 0], scalar=1.0,
                                               in1=h[:, c, ts].bitcast(f32),
                                               op0=MUL, op1=MUL)
                with nc.allow_low_precision(reason="bf16 rounding intentional"):
                    nc.vector.scalar_tensor_tensor(out=g[:, c, ts], in0=bc[:, 1],
                                                   scalar=bt[:, layer, c:c + 1],
                                                   in1=gtmp[:], op0=ADD, op1=ADD)


def _qkv_kv_exchange(nc, tc, dramp, g, qT, kT_all, qw, qkvb, layer):
    """QKV matmuls; k staged feature-major, v computed token-major; two
    AllGathers (k, then v) launched as soon as their inputs are staged."""
    bk_in = dramp.tile([D, TL], bf16, tag="bk_in")
    bk_out = dramp.tile([2, D, TL], bf16, tag="bk_out")
    bv_in = dramp.tile([KC, TL, P], bf16, tag="bv_in")
    bv_out = dramp.tile([2, KC, TL, P], bf16, tag="bv_out")
    with (
        tc.tile_pool(name="qkvps", bufs=3, space="PSUM") as qps,
        tc.tile_pool(name="vps", bufs=2, space="PSUM") as vps,
        tc.tile_pool(name="kstg", bufs=2) as kstg,
        tc.tile_pool(name="vstg", bufs=1) as vstg,
    ):
        # k chunks (pair-feature-major), staged + gathered first
        for c in range(KC):
            j = 4 + c
            ps = qps.tile([P, TL], f32, space="PSUM", tag="qkvp")
            for cc in range(KC):
                for t in range(2):
                    ts = slice(t * 512, (t + 1) * 512)
                    nc.tensor.matmul(ps[:, ts], qw[:, cc, j * P:(j + 1) * P],
                                     g[:, cc, ts], start=(cc == 0),
                                     stop=(cc == KC - 1))
            kst = kstg.tile([P, TL], bf16, tag="kst")
            with nc.allow_low_precision(reason="bf16 rounding intentional"):
                nc.scalar.activation(kst[:], ps[:], IDENT,
                                     bias=qkvb[:, layer, j:j + 1])
            nc.sync.dma_start(bk_in[c * P:(c + 1) * P, :], kst[:])
        if SIM_MODE or NO_COLLECTIVE:
            for half in range(2):
                nc.sync.dma_start(bk_out[half], bk_in[:])
        else:
            nc.gpsimd.collective_compute(
                "AllGather", mybir.AluOpType.bypass,
                replica_groups=[[0, 1], [2, 3], [4, 5], [6, 7]],
                ins=[bk_in.opt()], outs=[bk_out.opt()],
            )

        # v token-major: psum[tok, vfeat] = sum_c g[:,c,tokchunk]^T @ qw_v
        vst = vstg.tile([P, 8, KC, P], bf16, tag="vst")
        for tk in range(8):
            pv = vps.tile([P, 512], f32, space="PSUM", tag="vp")
            for cc in range(KC):
                nc.tensor.matmul(pv[:], g[:, cc, tk * P:(tk + 1) * P],
                                 qw[:, cc, 8 * P:12 * P],
                                 start=(cc == 0), stop=(cc == KC - 1))
            with nc.allow_low_precision(reason="bf16 rounding intentional"):
                nc.scalar.activation(
                    vst[:, tk].rearrange("p c f -> p (c f)"), pv[:], COPY)
        for c in range(KC):
            nc.sync.dma_start(
                bv_in[c].rearrange("(tk p) f -> p tk f", p=P), vst[:, :, c, :])
        if SIM_MODE or NO_COLLECTIVE:
            for half in range(2):
                nc.sync.dma_start(bv_out[half], bv_in[:])
        else:
            nc.gpsimd.collective_compute(
                "AllGather", mybir.AluOpType.bypass,
                replica_groups=[[0, 1], [2, 3], [4, 5], [6, 7]],
                ins=[bv_in.opt()], outs=[bv_out.opt()],
            )

        # q chunks last (attention pair c needs only chunk c)
        for j in range(4):
            ps = qps.tile([P, TL], f32, space="PSUM", tag="qkvp")
            for cc in range(KC):
                for t in range(2):
                    ts = slice(t * 512, (t + 1) * 512)
                    nc.tensor.matmul(ps[:, ts], qw[:, cc, j * P:(j + 1) * P],
                                     g[:, cc, ts], start=(cc == 0),
                                     stop=(cc == KC - 1))
            with nc.allow_low_precision(reason="bf16 rounding intentional"):
                nc.scalar.activation(qT[:, j], ps[:], IDENT,
                                     bias=qkvb[:, layer, j:j + 1])
    return bk_out, bv_out


def _attention(nc, tc, h, qT, oT, kT_all, recipt, sel,
               bk_out, bv_out, aw, qkvb, aob, layer):
    with (
        tc.tile_pool(name="attv", bufs=2) as vp,
        tc.tile_pool(name="attex", bufs=3) as exp_p,
        tc.tile_pool(name="attrb", bufs=2) as rbp,
        tc.tile_pool(name="attops", bufs=1, space="PSUM") as opsp,
        tc.tile_pool(name="attscs", bufs=2, space="PSUM") as scps,
    ):
        for c in range(H // 2):
            h1 = 2 * c
            nc.sync.dma_start(
                kT_all[:, c, :].rearrange("p (s t) -> p s t", s=2),
                bk_out[:, c * P:(c + 1) * P, :].rearrange("s p t -> p s t"))
            vaug = vp.tile([P, MC, 2, HD + 1], bf16, tag="vaug")
            nc.vector.memset(
                vaug[:, :, :, HD:HD + 1].rearrange("p m u o -> p (m u o)"), 1.0)
            for s in range(2):
                for u in range(2):
                    nc.sync.dma_start(
                        vaug[:, s * 8:(s + 1) * 8, u, 0:HD],
                        bv_out[s, c].rearrange("(m p) (u f) -> p m u f",
                                               p=P, u=2)[:, :, u, :])
            o_ps1 = opsp.tile([HD + 1, TL], f32, space="PSUM", tag="ops1",
                              name="ops1")
            o_ps2 = opsp.tile([HD + 1, TL], f32, space="PSUM", tag="ops2",
                              name="ops2")
            chunks = [(m, t) for m in range(MC) for t in range(2)]
            tiles = {}
            for i, (m, t) in enumerate(chunks):
                ts = slice(t * 512, (t + 1) * 512)
                s_ps = scps.tile([P, 2, 512], f32, space="PSUM", tag="sps")
                exps = exp_p.tile([P, 2, 512], bf16, tag="exps")
                nc.tensor.matmul(s_ps[:, 0], kT_all[0:HD, c, m * P:(m + 1) * P],
                                 qT[0:HD, c, ts], start=True, stop=True)
                nc.tensor.matmul(s_ps[:, 1], kT_all[HD:P, c, m * P:(m + 1) * P],
                                 qT[HD:P, c, ts], start=True, stop=True)
                with nc.allow_low_precision(reason="bf16 rounding intentional"):
                    nc.scalar.activation(exps[:], s_ps[:], EXP, scale=ATT_SCALE)
                tiles[i] = exps
                if i >= 1:
                    _av(nc, chunks, i - 1, tiles[i - 1], vaug, o_ps1, o_ps2)
                    del tiles[i - 1]
            _av(nc, chunks, len(chunks) - 1, tiles[len(chunks) - 1],
                vaug, o_ps1, o_ps2)
            with nc.allow_low_precision(reason="bf16 rounding intentional"):
                nc.vector.reciprocal(recipt[64:65, 0], o_ps1[64:65, :])
                nc.vector.reciprocal(recipt[64:65, 1], o_ps2[64:65, :])
            for t in range(2):
                ts = slice(t * 512, (t + 1) * 512)
                rbt = scps.tile([P, 2, 512], f32, space="PSUM", tag="sps")
                nc.tensor.matmul(rbt[0:HD, 0], sel[64:65, :], recipt[64:65, 0, ts],
                                 start=True, stop=True)
                nc.tensor.matmul(rbt[0:HD, 1], sel[64:65, :], recipt[64:65, 1, ts],
                                 start=True, stop=True)
                rbs = rbp.tile([P, 2, 512], bf16, tag="rbs")
                with nc.allow_low_precision(reason="bf16 rounding intentional"):
                    nc.vector.tensor_copy(rbs[0:HD], rbt[0:HD])
                    nc.vector.scalar_tensor_tensor(
                        out=oT[0:HD, c, ts], in0=o_ps1[0:HD, ts], scalar=1.0,
                        in1=rbs[0:HD, 0], op0=MUL, op1=MUL)
                    nc.vector.scalar_tensor_tensor(
                        out=oT[HD:P, c, ts], in0=o_ps2[0:HD, ts], scalar=1.0,
                        in1=rbs[0:HD, 1], op0=MUL, op1=MUL)
            with nc.allow_low_precision(reason="bf16 rounding intentional"):
                nc.vector.tensor_scalar(out=oT[:, c], in0=oT[:, c],
                                        scalar1=qkvb[:, layer, 8 + c:9 + c],
                                        scalar2=None, op0=ADD)
    with tc.tile_pool(name="aops", bufs=2, space="PSUM") as aops:
        for j in range(KC):
            ps = aops.tile([P, TL], f32, space="PSUM", tag="aop")
            for c in range(KC):
                for t in range(2):
                    ts = slice(t * 512, (t + 1) * 512)
                    nc.tensor.matmul(ps[:, ts], aw[:, c, j * P:(j + 1) * P],
                                     oT[:, c, ts], start=(c == 0),
                                     stop=(c == KC - 1))
            with nc.allow_low_precision(reason="f32r rounding intentional"):
                nc.vector.scalar_tensor_tensor(
                    out=h[:, j], in0=ps[:], scalar=aob[:, layer, j:j + 1],
                    in1=h[:, j].bitcast(f32), op0=ADD, op1=ADD)


def _av(nc, chunks, i, exps, vaug, o_ps1, o_ps2):
    m, t = chunks[i]
    ts = slice(t * 512, (t + 1) * 512)
    first = i == 0
    last = i == len(chunks) - 1
    # psum accumulation across (m, t) interleaves the two t-halves; each
    # column range accumulates only its own m's, start/stop on first/last i
    # touching that range: t halves are visited alternately so first two i's
    # are the starts and last two the stops.
    nc.tensor.matmul(o_ps1[:, ts], vaug[:, m, 0], exps[:, 0],
                     start=(i < 2), stop=(i >= len(chunks) - 2))
    nc.tensor.matmul(o_ps2[:, ts], vaug[:, m, 1], exps[:, 1],
                     start=(i < 2), stop=(i >= len(chunks) - 2))


def _ffn(nc, tc, h, g, f1w, f1b, f2w, f2b, layer):
    with (
        tc.tile_pool(name="zp", bufs=1) as zp,
        tc.tile_pool(name="f1ps", bufs=2, space="PSUM") as f1ps,
        tc.tile_pool(name="f2ps", bufs=1, space="PSUM") as f2ps,
    ):
        for th in range(2):
            ths = slice(th * 512, (th + 1) * 512)
            z = zp.tile([P, 16, 512], bf16, tag="z")
            for jp in range(8):
                ps = f1ps.tile([P, 2, 512], f32, space="PSUM", tag="f1p")
                for u in range(2):
                    j = 2 * jp + u
                    for c in range(KC):
                        nc.tensor.matmul(
                            ps[:, u], f1w[:, c, j * P:(j + 1) * P],
                            g[:, c, ths], start=(c == 0), stop=(c == KC - 1))
                with nc.allow_low_precision(reason="bf16 rounding intentional"):
                    nc.scalar.activation(z[:, 2 * jp], ps[:, 0], GELU,
                                         bias=f1b[:, layer, 2 * jp:2 * jp + 1])
                    nc.scalar.activation(z[:, 2 * jp + 1], ps[:, 1], GELU,
                                         bias=f1b[:, layer, 2 * jp + 1:2 * jp + 2])
            pso = [f2ps.tile([P, 512], f32, space="PSUM", tag=f"f2p{i}", name=f"f2p{i}")
                   for i in range(KC)]
            for k in range(16):
                for i in range(KC):
                    nc.tensor.matmul(pso[i][:], f2w[:, k, i * P:(i + 1) * P],
                                     z[:, k, :], start=(k == 0), stop=(k == 15))
            for i in range(KC):
                with nc.allow_low_precision(reason="f32r rounding intentional"):
                    nc.vector.scalar_tensor_tensor(
                        out=h[:, i, ths], in0=pso[i][:], scalar=f2b[:, layer, i:i + 1],
                        in1=h[:, i, ths].bitcast(f32), op0=ADD, op1=ADD)


def _logits(nc, tc, h, outw_in, logits_out):
    with (
        tc.tile_pool(name="lgsb", bufs=3) as lgsb,
        tc.tile_pool(name="lgw", bufs=1) as lgw,
        tc.tile_pool(name="lgps", bufs=3, space="PSUM") as lgps,
    ):
        oww = lgw.tile([P, KC, V], f32r, tag="oww")
        nc.sync.dma_start(oww[:], outw_in.rearrange("(c p) v -> p c v", p=P))
        for gq in range(8):
            ps = lgps.tile([P, V], f32, space="PSUM", tag="lgp")
            for c in range(KC):
                nc.tensor.matmul(ps[:], h[:, c, gq * P:(gq + 1) * P],
                                 oww[:, c], start=(c == 0), stop=(c == KC - 1))
            lg = lgsb.tile([P, V], f32, tag="lg")
            nc.vector.tensor_copy(lg[:], ps[:])
            nc.sync.dma_start(logits_out[gq * P:(gq + 1) * P, :], lg[:])


def _host_prep(inputs):
    bf = ml_dtypes.bfloat16
    emb = np.ascontiguousarray(inputs["emb"], dtype=np.float32)
    x = np.asarray(inputs["x"])
    ident = np.eye(P, dtype=np.float32)
    entw_row = np.ascontiguousarray(np.asarray(inputs["ent_w"])[:, 0], np.float32)
    entb_row = np.ascontiguousarray(np.asarray(inputs["ent_b"]), np.float32)
    qkv_wT = np.ascontiguousarray(np.transpose(inputs["qkv_w"], (0, 2, 1))).astype(bf)
    qkv_bt = np.ascontiguousarray(np.asarray(inputs["qkv_b"]).reshape(NL, 12, P).transpose(2, 0, 1), np.float32)
    ao_wT = np.ascontiguousarray(np.transpose(inputs["ao_w"], (0, 2, 1))).astype(bf)
    ao_bt = np.ascontiguousarray(np.asarray(inputs["ao_b"]).reshape(NL, KC, P).transpose(2, 0, 1), np.float32)
    ff1_wT = np.ascontiguousarray(np.transpose(inputs["ff1_w"], (0, 2, 1))).astype(bf)
    ff1_bt = np.ascontiguousarray(np.asarray(inputs["ff1_b"]).reshape(NL, 16, P).transpose(2, 0, 1), np.float32)
    ff2_wT = np.ascontiguousarray(np.transpose(inputs["ff2_w"], (0, 2, 1))).astype(bf)
    ff2_bt = np.ascontiguousarray(np.asarray(inputs["ff2_b"]).reshape(NL, KC, P).transpose(2, 0, 1), np.float32)
    ln1_srow = np.ascontiguousarray(np.asarray(inputs["ln1_s"]).reshape(NL, KC * P)).astype(bf)
    ln1_bt = np.ascontiguousarray(np.asarray(inputs["ln1_b"]).reshape(NL, KC, P).transpose(2, 0, 1), np.float32)
    ln2_srow = np.ascontiguousarray(np.asarray(inputs["ln2_s"]).reshape(NL, KC * P)).astype(bf)
    ln2_bt = np.ascontiguousarray(np.asarray(inputs["ln2_b"]).reshape(NL, KC, P).transpose(2, 0, 1), np.float32)
    out_wT = np.ascontiguousarray(np.asarray(inputs["out_w"]).T, np.float32)

    shared = dict(emb=emb, ident=ident, entw_row=entw_row, entb_row=entb_row,
                  qkv_wT=qkv_wT, qkv_bt=qkv_bt, ao_wT=ao_wT, ao_bt=ao_bt,
                  ff1_wT=ff1_wT, ff1_bt=ff1_bt, ff2_wT=ff2_wT, ff2_bt=ff2_bt,
                  ln1_srow=ln1_srow, ln1_bt=ln1_bt, ln2_srow=ln2_srow,
                  ln2_bt=ln2_bt, out_wT=out_wT)
    in_maps = []
    for core in range(NCORE):
        b, s = divmod(core, 2)
        t0 = s * TL
        xb = np.asarray(x[b], dtype=np.int64)
        x_idx = xb[t0:t0 + TL].astype(np.int32)
        xpad = np.concatenate([xb, np.zeros(8, np.int64)])
        x_ent = xpad[t0:t0 + 1032].astype(np.float32)
        pos = np.arange(t0, t0 + TL)
        ent_mask = (pos <= L - W).astype(np.float32)
        in_maps.append(dict(shared, x_idx=x_idx, x_ent=x_ent, ent_mask=ent_mask))
    return in_maps


def kernel(**inputs) -> np.ndarray:
    nc = build_program(1)
    in_maps = _host_prep(inputs)
    res = run_bass_kernel_spmd(nc, in_maps, list(range(NCORE)))
    logits = np.empty((B, L, V), np.float32)
    for core in range(NCORE):
        b, s = divmod(core, 2)
        logits[b, s * TL:(s + 1) * TL, :] = res.results[core]["logits"]
    return logits
